# revision 8
# baseline (speedup 1.0000x reference)
"""Continuous Game-of-Life Trainium2 kernel.

Reference computation (per batch image, cyclic 3x3 stencil):
    around = 8-neighbor sum of x (torus wrap)
    survive = sigmoid(10(around-1.5)) * sigmoid(10(3.5-around))
    birth   = sigmoid(10(around-2.5)) * sigmoid(10(3.5-around))
    out     = x*survive + (1-x)*birth

Algebraic simplification used here: with BETA=10 the sigmoid transitions at
1.5/2.5/3.5 are >= 1.0 apart, so sigmoid products collapse into differences:
    s_c := sigmoid(10*around - 10*c)
    survive ~= s1.5 - s3.5   (error ~2e-9)
    birth   ~= s2.5 - s3.5   (error ~4.5e-5)
    out ~= x*(s1.5 - s2.5) + (s2.5 - s3.5)    (max abs err 4.5e-5)

Mapping to engines (per 126-row strip of a 2048x2048 image):
  - DMA: load 128 input rows (1 halo row above/below) as fp32.
  - TensorE: the whole 8-neighbor sum via 3 banded matmuls accumulated in
    PSUM: tridiag-no-center band (vertical neighbors, center column) plus
    full tridiag band applied to column-shifted views (left/right columns);
    W-wrap handled by two 1-column edge matmuls.
  - ScalarE: the three sigmoids straight out of PSUM (scale/bias fused).
  - VectorE/GpSimd: 4 cheap elementwise ops, fp16 (2x DVE mode).
  - DMA out: fp16 (host upcasts to fp32).

Sharding: pure data-parallel over batch: 16 images -> 8 cores x 2 images.
The torus wrap is per-image so there is no cross-core halo at all.
"""

import numpy as np

B, H, W = 16, 2048, 2048
N_CORES = 8
B_PER = B // N_CORES  # 2 images per core
STRIDE = 126  # output rows per strip (128 input rows incl. 1-row halos)
N_STRIPS = (H + STRIDE - 1) // STRIDE  # 17
NBANKS = W // 512  # PSUM banks per strip

_cached_nc = None


def _band_matrices(stride=STRIDE):
    """[stride+2, stride] fp32 stationary operands.

    m0[k, p] = 1 iff k in {p, p+2}     (vertical neighbors, no center)
    m1[k, p] = 1 iff k in {p, p+1, p+2} (full 3-tap)

    Output row p of a strip is image row stride*t + p, which is input-tile
    row p+1; its vertical neighbors are tile rows p and p+2.
    """
    m0 = np.zeros((stride + 2, stride), np.float32)
    m1 = np.zeros((stride + 2, stride), np.float32)
    for p in range(stride):
        m0[p, p] = 1.0
        m0[p + 2, p] = 1.0
        m1[p, p] = 1.0
        m1[p + 1, p] = 1.0
        m1[p + 2, p] = 1.0
    return m0, m1


def _build(b_per=B_PER, h=H, w=W, stride=STRIDE):
    global _cached_nc
    if _cached_nc is not None and (b_per, h, w, stride) == (B_PER, H, W, STRIDE):
        return _cached_nc

    import concourse.mybir as mybir
    from concourse.bacc import Bacc
    from concourse.tile import TileContext

    B_PER_, H_, W_, STRIDE_ = b_per, h, w, stride
    N_STRIPS_ = (H_ + STRIDE_ - 1) // STRIDE_
    NBANKS_ = W_ // 512
    KROWS = STRIDE_ + 2  # input rows per full strip

    f32 = mybir.dt.float32
    f16 = mybir.dt.float16
    Sig = mybir.ActivationFunctionType.Sigmoid

    nc = Bacc(trn_type="TRN2")
    x_d = nc.dram_tensor("x", [B_PER_, H_, W_], f32, kind="ExternalInput")
    y_d = nc.dram_tensor("y", [B_PER_, H_, W_], f16, kind="ExternalOutput")

    m0_np, m1_np = _band_matrices(STRIDE_)
    m0_d = nc.inline_tensor(m0_np, "m0_const")
    m1_d = nc.inline_tensor(m1_np, "m1_const")

    with TileContext(nc) as tc:
        with (
            tc.tile_pool(name="wpool", bufs=1) as wpool,
            tc.tile_pool(name="xpool", bufs=4) as xpool,
            tc.tile_pool(name="spool", bufs=3) as spool,
            tc.tile_pool(name="tpool", bufs=3) as tpool,
            tc.tile_pool(name="opool", bufs=4) as opool,
            tc.tile_pool(name="ppool", bufs=2, space="PSUM") as ppool,
        ):
            m0 = wpool.tile([KROWS, STRIDE_], f32)
            m1 = wpool.tile([KROWS, STRIDE_], f32)
            nc.sync.dma_start(out=m0[:], in_=m0_d[:])
            nc.sync.dma_start(out=m1[:], in_=m1_d[:])

            # activation biases must be [128,1] APs, not immediates
            b15 = wpool.tile([128, 1], f32)
            b25 = wpool.tile([128, 1], f32)
            b35 = wpool.tile([128, 1], f32)
            nc.vector.memset(b15[:], -15.0)
            nc.vector.memset(b25[:], -25.0)
            nc.vector.memset(b35[:], -35.0)

            for b in range(B_PER_):
                for t in range(N_STRIPS_):
                    r0 = t * STRIDE_  # first output row
                    M = min(STRIDE_, H_ - r0)  # output rows this strip
                    k = M + 2  # input rows used (incl halos)
                    ri = r0 - 1  # first input row (may wrap)

                    # Wrap strips need two DMAs; issue those on SWDGE
                    # (gpsimd) so both land on the single DMASW0 semaphore
                    # lane -- a consumer may carry at most 2 sync waits, and
                    # the first matmul of a strip also waits on the PSUM
                    # buffer release.
                    xt = xpool.tile([KROWS, W_], f32, tag="xt")
                    if ri < 0:
                        nc.gpsimd.dma_start(out=xt[0:1, :], in_=x_d[b, H_ - 1 : H_, :])
                        nc.gpsimd.dma_start(out=xt[1:k, :], in_=x_d[b, 0 : k - 1, :])
                    elif ri + k > H_:
                        n1 = H_ - ri
                        nc.gpsimd.dma_start(out=xt[0:n1, :], in_=x_d[b, ri:H_, :])
                        nc.gpsimd.dma_start(out=xt[n1:k, :], in_=x_d[b, 0 : k - n1, :])
                    else:
                        nc.sync.dma_start(out=xt[0:k, :], in_=x_d[b, ri : ri + k, :])

                    ps = ppool.tile([STRIDE_, W_], f32, tag="ps")
                    m0s = m0[:k, :M]
                    m1s = m1[:k, :M]

                    # around = sum of 8 neighbors, accumulated in PSUM.
                    for nb in range(NBANKS_):
                        c0 = nb * 512
                        c1 = c0 + 512
                        # center column, vertical neighbors only
                        nc.tensor.matmul(
                            ps[:M, c0:c1], m0s, xt[:k, c0:c1],
                            start=True, stop=False,
                        )
                        # left-neighbor column: out col j += band @ x col j-1
                        if nb == 0:
                            nc.tensor.matmul(
                                ps[:M, 1:512], m1s, xt[:k, 0:511],
                                start=False, stop=False,
                            )
                            nc.tensor.matmul(
                                ps[:M, 0:1], m1s, xt[:k, W_ - 1 : W_],
                                start=False, stop=False,
                            )
                        else:
                            nc.tensor.matmul(
                                ps[:M, c0:c1], m1s, xt[:k, c0 - 1 : c1 - 1],
                                start=False, stop=False,
                            )
                        # right-neighbor column: out col j += band @ x col j+1
                        if nb == NBANKS_ - 1:
                            nc.tensor.matmul(
                                ps[:M, c0 : W_ - 1], m1s, xt[:k, c0 + 1 : W_],
                                start=False, stop=False,
                            )
                            nc.tensor.matmul(
                                ps[:M, W_ - 1 : W_], m1s, xt[:k, 0:1],
                                start=False, stop=True,
                            )
                        else:
                            nc.tensor.matmul(
                                ps[:M, c0:c1], m1s, xt[:k, c0 + 1 : c1 + 1],
                                start=False, stop=True,
                            )

                    s15 = spool.tile([STRIDE_, W_], f16, tag="s15")
                    s25 = spool.tile([STRIDE_, W_], f16, tag="s25")
                    s35 = spool.tile([STRIDE_, W_], f16, tag="s35")
                    nc.scalar.activation(s15[:M], ps[:M], Sig, bias=b15[:M], scale=10.0)
                    nc.scalar.activation(s25[:M], ps[:M], Sig, bias=b25[:M], scale=10.0)
                    nc.scalar.activation(s35[:M], ps[:M], Sig, bias=b35[:M], scale=10.0)

                    # cell term: x rows r0..r0+M live at tile partitions
                    # 1..M+1; compute engines can't address a partition-1
                    # base, so shift down (and cast to fp16) with a
                    # SBUF->SBUF DMA.
                    xc = tpool.tile([STRIDE_, W_], f16, tag="xc")
                    nc.gpsimd.dma_start(out=xc[:M], in_=xt[1 : M + 1, :])

                    d = tpool.tile([STRIDE_, W_], f16, tag="d")
                    e = tpool.tile([STRIDE_, W_], f16, tag="e")
                    m = tpool.tile([STRIDE_, W_], f16, tag="m")
                    o = opool.tile([STRIDE_, W_], f16, tag="o")
                    nc.vector.tensor_sub(out=d[:M], in0=s15[:M], in1=s25[:M])
                    nc.vector.tensor_sub(out=e[:M], in0=s25[:M], in1=s35[:M])
                    nc.vector.tensor_mul(out=m[:M], in0=xc[:M], in1=d[:M])
                    nc.vector.tensor_add(out=o[:M], in0=m[:M], in1=e[:M])

                    nc.sync.dma_start(out=y_d[b, r0 : r0 + M, :], in_=o[:M])

    nc.compile()
    if (b_per, h, w, stride) == (B_PER, H, W, STRIDE):
        _cached_nc = nc
    return nc


def run(x, trace=False):
    """Run the SPMD kernel on 8 cores. Returns (out_fp32, BassKernelResults)."""
    from concourse.bass_utils import run_bass_kernel_spmd

    nc = _build()
    x = np.asarray(x, dtype=np.float32)
    assert x.shape == (B, H, W), x.shape
    in_maps = [{"x": x[B_PER * c : B_PER * (c + 1)]} for c in range(N_CORES)]
    res = run_bass_kernel_spmd(nc, in_maps, core_ids=list(range(N_CORES)), trace=trace)
    out = np.concatenate(
        [res.results[c]["y"].astype(np.float32) for c in range(N_CORES)], axis=0
    )
    return out, res


def kernel(x):
    out, _ = run(x, trace=False)
    return out


# revision 9
# speedup vs baseline: 189.4323x; 189.4323x over previous
"""Continuous Game-of-Life Trainium2 kernel.

Reference computation (per batch image, cyclic 3x3 stencil):
    around = 8-neighbor sum of x (torus wrap)
    survive = sigmoid(10(around-1.5)) * sigmoid(10(3.5-around))
    birth   = sigmoid(10(around-2.5)) * sigmoid(10(3.5-around))
    out     = x*survive + (1-x)*birth

Algebraic simplification used here: with BETA=10 the sigmoid transitions at
1.5/2.5/3.5 are >= 1.0 apart, so sigmoid products collapse into differences:
    s_c := sigmoid(10*around - 10*c)
    survive ~= s1.5 - s3.5   (error ~2e-9)
    birth   ~= s2.5 - s3.5   (error ~4.5e-5)
    out ~= x*(s1.5 - s2.5) + (s2.5 - s3.5)    (max abs err 4.5e-5)

Mapping to engines (per 126-row strip of a 2048x2048 image):
  - DMA: load 128 input rows (1 halo row above/below) as fp32.
  - TensorE: the whole 8-neighbor sum via 3 banded matmuls accumulated in
    PSUM: tridiag-no-center band (vertical neighbors, center column) plus
    full tridiag band applied to column-shifted views (left/right columns);
    W-wrap handled by two 1-column edge matmuls.
  - ScalarE: the three sigmoids straight out of PSUM (scale/bias fused).
  - VectorE/GpSimd: 4 cheap elementwise ops, fp16 (2x DVE mode).
  - DMA out: fp16 (host upcasts to fp32).

Sharding: pure data-parallel over batch: 16 images -> 8 cores x 2 images.
The torus wrap is per-image so there is no cross-core halo at all.
"""

import numpy as np

B, H, W = 16, 2048, 2048
N_CORES = 8
B_PER = B // N_CORES  # 2 images per core
STRIDE = 126  # output rows per strip (128 input rows incl. 1-row halos)
N_STRIPS = (H + STRIDE - 1) // STRIDE  # 17
NBANKS = W // 512  # PSUM banks per strip

_cached_nc = None


def _band_matrices(stride=STRIDE):
    """[stride+2, stride] fp32 stationary operands.

    m0[k, p] = 1 iff k in {p, p+2}     (vertical neighbors, no center)
    m1[k, p] = 1 iff k in {p, p+1, p+2} (full 3-tap)

    Output row p of a strip is image row stride*t + p, which is input-tile
    row p+1; its vertical neighbors are tile rows p and p+2.
    """
    m0 = np.zeros((stride + 2, stride), np.float32)
    m1 = np.zeros((stride + 2, stride), np.float32)
    for p in range(stride):
        m0[p, p] = 1.0
        m0[p + 2, p] = 1.0
        m1[p, p] = 1.0
        m1[p + 1, p] = 1.0
        m1[p + 2, p] = 1.0
    return m0, m1


def _build(b_per=B_PER, h=H, w=W, stride=STRIDE, repeat=1):
    global _cached_nc
    if _cached_nc is not None and (b_per, h, w, stride, repeat) == (
        B_PER, H, W, STRIDE, 1
    ):
        return _cached_nc

    import concourse.mybir as mybir
    from concourse.bacc import Bacc
    from concourse.tile import TileContext

    B_PER_, H_, W_, STRIDE_ = b_per, h, w, stride
    N_STRIPS_ = (H_ + STRIDE_ - 1) // STRIDE_
    NBANKS_ = W_ // 512
    KROWS = STRIDE_ + 2  # input rows per full strip

    f32 = mybir.dt.float32
    f16 = mybir.dt.float16
    Sig = mybir.ActivationFunctionType.Sigmoid

    nc = Bacc(trn_type="TRN2")
    x_d = nc.dram_tensor("x", [B_PER_, H_, W_], f32, kind="ExternalInput")
    y_d = nc.dram_tensor("y", [B_PER_, H_, W_], f16, kind="ExternalOutput")

    m0_np, m1_np = _band_matrices(STRIDE_)
    m0_d = nc.inline_tensor(m0_np, "m0_const")
    m1_d = nc.inline_tensor(m1_np, "m1_const")

    with TileContext(nc) as tc:
        with (
            tc.tile_pool(name="wpool", bufs=1) as wpool,
            tc.tile_pool(name="xpool", bufs=4) as xpool,
            tc.tile_pool(name="spool", bufs=3) as spool,
            tc.tile_pool(name="tpool", bufs=3) as tpool,
            tc.tile_pool(name="opool", bufs=4) as opool,
            tc.tile_pool(name="ppool", bufs=2, space="PSUM") as ppool,
        ):
            m0 = wpool.tile([KROWS, STRIDE_], f32)
            m1 = wpool.tile([KROWS, STRIDE_], f32)
            nc.sync.dma_start(out=m0[:], in_=m0_d[:])
            nc.sync.dma_start(out=m1[:], in_=m1_d[:])

            # activation biases must be [128,1] APs, not immediates
            b15 = wpool.tile([128, 1], f32)
            b25 = wpool.tile([128, 1], f32)
            b35 = wpool.tile([128, 1], f32)
            nc.vector.memset(b15[:], -15.0)
            nc.vector.memset(b25[:], -25.0)
            nc.vector.memset(b35[:], -35.0)

            for rep in range(repeat):
              for b in range(B_PER_):
                for t in range(N_STRIPS_):
                    r0 = t * STRIDE_  # first output row
                    M = min(STRIDE_, H_ - r0)  # output rows this strip
                    k = M + 2  # input rows used (incl halos)
                    ri = r0 - 1  # first input row (may wrap)

                    # Wrap strips need two DMAs; issue those on SWDGE
                    # (gpsimd) so both land on the single DMASW0 semaphore
                    # lane -- a consumer may carry at most 2 sync waits, and
                    # the first matmul of a strip also waits on the PSUM
                    # buffer release.
                    xt = xpool.tile([KROWS, W_], f32, tag="xt")
                    if ri < 0:
                        nc.gpsimd.dma_start(out=xt[0:1, :], in_=x_d[b, H_ - 1 : H_, :])
                        nc.gpsimd.dma_start(out=xt[1:k, :], in_=x_d[b, 0 : k - 1, :])
                    elif ri + k > H_:
                        n1 = H_ - ri
                        nc.gpsimd.dma_start(out=xt[0:n1, :], in_=x_d[b, ri:H_, :])
                        nc.gpsimd.dma_start(out=xt[n1:k, :], in_=x_d[b, 0 : k - n1, :])
                    else:
                        nc.sync.dma_start(out=xt[0:k, :], in_=x_d[b, ri : ri + k, :])

                    ps = ppool.tile([STRIDE_, W_], f32, tag="ps")
                    m0s = m0[:k, :M]
                    m1s = m1[:k, :M]

                    # around = sum of 8 neighbors, accumulated in PSUM.
                    for nb in range(NBANKS_):
                        c0 = nb * 512
                        c1 = c0 + 512
                        # center column, vertical neighbors only
                        nc.tensor.matmul(
                            ps[:M, c0:c1], m0s, xt[:k, c0:c1],
                            start=True, stop=False,
                        )
                        # left-neighbor column: out col j += band @ x col j-1
                        if nb == 0:
                            nc.tensor.matmul(
                                ps[:M, 1:512], m1s, xt[:k, 0:511],
                                start=False, stop=False,
                            )
                            nc.tensor.matmul(
                                ps[:M, 0:1], m1s, xt[:k, W_ - 1 : W_],
                                start=False, stop=False,
                            )
                        else:
                            nc.tensor.matmul(
                                ps[:M, c0:c1], m1s, xt[:k, c0 - 1 : c1 - 1],
                                start=False, stop=False,
                            )
                        # right-neighbor column: out col j += band @ x col j+1
                        if nb == NBANKS_ - 1:
                            nc.tensor.matmul(
                                ps[:M, c0 : W_ - 1], m1s, xt[:k, c0 + 1 : W_],
                                start=False, stop=False,
                            )
                            nc.tensor.matmul(
                                ps[:M, W_ - 1 : W_], m1s, xt[:k, 0:1],
                                start=False, stop=True,
                            )
                        else:
                            nc.tensor.matmul(
                                ps[:M, c0:c1], m1s, xt[:k, c0 + 1 : c1 + 1],
                                start=False, stop=True,
                            )

                    s15 = spool.tile([STRIDE_, W_], f16, tag="s15")
                    s25 = spool.tile([STRIDE_, W_], f16, tag="s25")
                    s35 = spool.tile([STRIDE_, W_], f16, tag="s35")
                    nc.scalar.activation(s15[:M], ps[:M], Sig, bias=b15[:M], scale=10.0)
                    nc.scalar.activation(s25[:M], ps[:M], Sig, bias=b25[:M], scale=10.0)
                    nc.scalar.activation(s35[:M], ps[:M], Sig, bias=b35[:M], scale=10.0)

                    # cell term: x rows r0..r0+M live at tile partitions
                    # 1..M+1; compute engines can't address a partition-1
                    # base, so shift down (and cast to fp16) with a
                    # SBUF->SBUF DMA.
                    xc = tpool.tile([STRIDE_, W_], f16, tag="xc")
                    nc.gpsimd.dma_start(out=xc[:M], in_=xt[1 : M + 1, :])

                    d = tpool.tile([STRIDE_, W_], f16, tag="d")
                    e = tpool.tile([STRIDE_, W_], f16, tag="e")
                    m = tpool.tile([STRIDE_, W_], f16, tag="m")
                    o = opool.tile([STRIDE_, W_], f16, tag="o")
                    nc.vector.tensor_sub(out=d[:M], in0=s15[:M], in1=s25[:M])
                    nc.vector.tensor_sub(out=e[:M], in0=s25[:M], in1=s35[:M])
                    nc.vector.tensor_mul(out=m[:M], in0=xc[:M], in1=d[:M])
                    nc.vector.tensor_add(out=o[:M], in0=m[:M], in1=e[:M])

                    nc.sync.dma_start(out=y_d[b, r0 : r0 + M, :], in_=o[:M])

    nc.compile()
    if (b_per, h, w, stride, repeat) == (B_PER, H, W, STRIDE, 1):
        _cached_nc = nc
    return nc


def run(x, trace=False):
    """Run the SPMD kernel on 8 cores. Returns (out_fp32, BassKernelResults)."""
    from concourse.bass_utils import run_bass_kernel_spmd

    nc = _build()
    x = np.asarray(x, dtype=np.float32)
    assert x.shape == (B, H, W), x.shape
    in_maps = [{"x": x[B_PER * c : B_PER * (c + 1)]} for c in range(N_CORES)]
    res = run_bass_kernel_spmd(nc, in_maps, core_ids=list(range(N_CORES)), trace=trace)
    out = np.concatenate(
        [res.results[c]["y"].astype(np.float32) for c in range(N_CORES)], axis=0
    )
    return out, res


def kernel(x):
    out, _ = run(x, trace=False)
    return out


# revision 10
# speedup vs baseline: 236.6155x; 1.2491x over previous
"""Continuous Game-of-Life Trainium2 kernel.

Reference computation (per batch image, cyclic 3x3 stencil):
    around = 8-neighbor sum of x (torus wrap)
    survive = sigmoid(10(around-1.5)) * sigmoid(10(3.5-around))
    birth   = sigmoid(10(around-2.5)) * sigmoid(10(3.5-around))
    out     = x*survive + (1-x)*birth

Algebraic simplification used here: with BETA=10 the sigmoid transitions at
1.5/2.5/3.5 are >= 1.0 apart, so sigmoid products collapse into differences:
    s_c := sigmoid(10*around - 10*c)
    survive ~= s1.5 - s3.5   (error ~2e-9)
    birth   ~= s2.5 - s3.5   (error ~4.5e-5)
    out ~= x*(s1.5 - s2.5) + (s2.5 - s3.5)    (max abs err 4.5e-5)

Mapping to engines (per 126-row strip of a 2048x2048 image):
  - DMA: load 128 input rows (1 halo row above/below) as fp32.
  - TensorE: the whole 8-neighbor sum via 3 banded matmuls accumulated in
    PSUM: tridiag-no-center band (vertical neighbors, center column) plus
    full tridiag band applied to column-shifted views (left/right columns);
    W-wrap handled by two 1-column edge matmuls.
  - ScalarE: the three sigmoids straight out of PSUM (scale/bias fused).
  - VectorE/GpSimd: 4 cheap elementwise ops, fp16 (2x DVE mode).
  - DMA out: fp16 (host upcasts to fp32).

Sharding: pure data-parallel over batch: 16 images -> 8 cores x 2 images.
The torus wrap is per-image so there is no cross-core halo at all.
"""

import numpy as np

B, H, W = 16, 2048, 2048
N_CORES = 8
B_PER = B // N_CORES  # 2 images per core
STRIDE = 126  # output rows per strip (128 input rows incl. 1-row halos)
N_STRIPS = (H + STRIDE - 1) // STRIDE  # 17
NBANKS = W // 512  # PSUM banks per strip

_cached_nc = None


def _band_matrices(m, dtype=np.float16):
    """[m+2, m] stationary operands for the vertical taps.

    Tile layout: partitions 0..m-1 hold image rows r0..r0+m-1 (the cells),
    partition m holds the bottom halo row r0+m, partition m+1 holds the top
    halo row r0-1.  For output row p the vertical neighbors are partitions
    p-1 (or m+1 when p==0) and p+1.

    m0[k, p] = 1 for the two vertical neighbors (no center),
    m1[k, p] = 1 for the full 3-tap (used on the column-shifted views).
    """
    m0 = np.zeros((m + 2, m), dtype)
    m1 = np.zeros((m + 2, m), dtype)
    for p in range(m):
        up = m + 1 if p == 0 else p - 1
        m0[up, p] = 1.0
        m0[p + 1, p] = 1.0
        m1[up, p] = 1.0
        m1[p, p] = 1.0
        m1[p + 1, p] = 1.0
    return m0, m1


def _build(b_per=B_PER, h=H, w=W, stride=STRIDE, repeat=1):
    global _cached_nc
    if _cached_nc is not None and (b_per, h, w, stride, repeat) == (
        B_PER, H, W, STRIDE, 1
    ):
        return _cached_nc

    import concourse.mybir as mybir
    from concourse.bacc import Bacc
    from concourse.tile import TileContext

    B_PER_, H_, W_, STRIDE_ = b_per, h, w, stride
    N_STRIPS_ = (H_ + STRIDE_ - 1) // STRIDE_
    NBANKS_ = W_ // 512
    KROWS = STRIDE_ + 2  # input rows per full strip

    f32 = mybir.dt.float32
    f16 = mybir.dt.float16
    Sig = mybir.ActivationFunctionType.Sigmoid

    nc = Bacc(trn_type="TRN2")
    x_d = nc.dram_tensor("x", [B_PER_, H_, W_], f32, kind="ExternalInput")
    y_d = nc.dram_tensor("y", [B_PER_, H_, W_], f16, kind="ExternalOutput")

    consts = {}
    for m in sorted({STRIDE_, H_ - STRIDE_ * (N_STRIPS_ - 1)}):
        m0_np, m1_np = _band_matrices(m)
        consts[m] = (
            nc.inline_tensor(m0_np, f"m0_const_{m}"),
            nc.inline_tensor(m1_np, f"m1_const_{m}"),
        )

    with TileContext(nc) as tc:
        with (
            tc.tile_pool(name="wpool", bufs=1) as wpool,
            tc.tile_pool(name="xpool", bufs=4) as xpool,
            tc.tile_pool(name="spool", bufs=3) as spool,
            tc.tile_pool(name="tpool", bufs=3) as tpool,
            tc.tile_pool(name="opool", bufs=4) as opool,
            tc.tile_pool(name="ppool", bufs=2, space="PSUM") as ppool,
        ):
            bands = {}
            for m, (m0_d, m1_d) in consts.items():
                m0 = wpool.tile([m + 2, m], f16, name=f"m0_{m}")
                m1 = wpool.tile([m + 2, m], f16, name=f"m1_{m}")
                nc.sync.dma_start(out=m0[:], in_=m0_d[:])
                nc.sync.dma_start(out=m1[:], in_=m1_d[:])
                bands[m] = (m0, m1)

            # activation biases must be [128,1] APs, not immediates
            b15 = wpool.tile([128, 1], f32)
            b25 = wpool.tile([128, 1], f32)
            b35 = wpool.tile([128, 1], f32)
            nc.vector.memset(b15[:], -15.0)
            nc.vector.memset(b25[:], -25.0)
            nc.vector.memset(b35[:], -35.0)

            for rep in range(repeat):
              for b in range(B_PER_):
                for t in range(N_STRIPS_):
                    r0 = t * STRIDE_  # first output row
                    M = min(STRIDE_, H_ - r0)  # output rows this strip
                    k = M + 2  # partitions used (cells + 2 halos)
                    m0, m1 = bands[M]

                    # fp16 tile, partitions 0..M-1 = cells (rows r0..),
                    # partition M = bottom halo, M+1 = top halo.  gpsimd
                    # (SWDGE) DMA casts fp32->fp16 in flight.
                    xt = xpool.tile([KROWS, W_], f16, tag="xt")
                    if r0 + M < H_:
                        # cells + bottom halo contiguous
                        nc.gpsimd.dma_start(
                            out=xt[0 : M + 1, :], in_=x_d[b, r0 : r0 + M + 1, :]
                        )
                    else:
                        # last strip: bottom halo wraps to row 0
                        nc.gpsimd.dma_start(out=xt[0:M, :], in_=x_d[b, r0:H_, :])
                        nc.gpsimd.dma_start(out=xt[M : M + 1, :], in_=x_d[b, 0:1, :])
                    rtop = (r0 - 1) % H_
                    nc.gpsimd.dma_start(
                        out=xt[M + 1 : M + 2, :], in_=x_d[b, rtop : rtop + 1, :]
                    )

                    ps = ppool.tile([STRIDE_, W_], f32, tag="ps")
                    m0s = m0[:k, :M]
                    m1s = m1[:k, :M]

                    # around = sum of 8 neighbors, accumulated in PSUM.
                    for nb in range(NBANKS_):
                        c0 = nb * 512
                        c1 = c0 + 512
                        # center column, vertical neighbors only
                        nc.tensor.matmul(
                            ps[:M, c0:c1], m0s, xt[:k, c0:c1],
                            start=True, stop=False,
                        )
                        # left-neighbor column: out col j += band @ x col j-1
                        if nb == 0:
                            nc.tensor.matmul(
                                ps[:M, 1:512], m1s, xt[:k, 0:511],
                                start=False, stop=False,
                            )
                            nc.tensor.matmul(
                                ps[:M, 0:1], m1s, xt[:k, W_ - 1 : W_],
                                start=False, stop=False,
                            )
                        else:
                            nc.tensor.matmul(
                                ps[:M, c0:c1], m1s, xt[:k, c0 - 1 : c1 - 1],
                                start=False, stop=False,
                            )
                        # right-neighbor column: out col j += band @ x col j+1
                        if nb == NBANKS_ - 1:
                            nc.tensor.matmul(
                                ps[:M, c0 : W_ - 1], m1s, xt[:k, c0 + 1 : W_],
                                start=False, stop=False,
                            )
                            nc.tensor.matmul(
                                ps[:M, W_ - 1 : W_], m1s, xt[:k, 0:1],
                                start=False, stop=True,
                            )
                        else:
                            nc.tensor.matmul(
                                ps[:M, c0:c1], m1s, xt[:k, c0 + 1 : c1 + 1],
                                start=False, stop=True,
                            )

                    s15 = spool.tile([STRIDE_, W_], f16, tag="s15")
                    s25 = spool.tile([STRIDE_, W_], f16, tag="s25")
                    s35 = spool.tile([STRIDE_, W_], f16, tag="s35")
                    nc.scalar.activation(s15[:M], ps[:M], Sig, bias=b15[:M], scale=10.0)
                    nc.scalar.activation(s25[:M], ps[:M], Sig, bias=b25[:M], scale=10.0)
                    nc.scalar.activation(s35[:M], ps[:M], Sig, bias=b35[:M], scale=10.0)

                    d = tpool.tile([STRIDE_, W_], f16, tag="d")
                    e = tpool.tile([STRIDE_, W_], f16, tag="e")
                    m = tpool.tile([STRIDE_, W_], f16, tag="m")
                    o = opool.tile([STRIDE_, W_], f16, tag="o")
                    nc.vector.tensor_sub(out=d[:M], in0=s15[:M], in1=s25[:M])
                    nc.vector.tensor_sub(out=e[:M], in0=s25[:M], in1=s35[:M])
                    nc.vector.tensor_mul(out=m[:M], in0=xt[:M, :], in1=d[:M])
                    nc.vector.tensor_add(out=o[:M], in0=m[:M], in1=e[:M])

                    nc.sync.dma_start(out=y_d[b, r0 : r0 + M, :], in_=o[:M])

    nc.compile()
    if (b_per, h, w, stride, repeat) == (B_PER, H, W, STRIDE, 1):
        _cached_nc = nc
    return nc


def run(x, trace=False):
    """Run the SPMD kernel on 8 cores. Returns (out_fp32, BassKernelResults)."""
    from concourse.bass_utils import run_bass_kernel_spmd

    nc = _build()
    x = np.asarray(x, dtype=np.float32)
    assert x.shape == (B, H, W), x.shape
    in_maps = [{"x": x[B_PER * c : B_PER * (c + 1)]} for c in range(N_CORES)]
    res = run_bass_kernel_spmd(nc, in_maps, core_ids=list(range(N_CORES)), trace=trace)
    out = np.concatenate(
        [res.results[c]["y"].astype(np.float32) for c in range(N_CORES)], axis=0
    )
    return out, res


def kernel(x):
    out, _ = run(x, trace=False)
    return out


# revision 16
# speedup vs baseline: 353.3891x; 1.4935x over previous
"""Continuous Game-of-Life Trainium2 kernel.

Reference computation (per batch image, cyclic 3x3 stencil):
    around = 8-neighbor sum of x (torus wrap)
    survive = sigmoid(10(around-1.5)) * sigmoid(10(3.5-around))
    birth   = sigmoid(10(around-2.5)) * sigmoid(10(3.5-around))
    out     = x*survive + (1-x)*birth

Algebraic simplification used here: with BETA=10 the sigmoid transitions at
1.5/2.5/3.5 are >= 1.0 apart, so sigmoid products collapse into differences:
    s_c := sigmoid(10*around - 10*c)
    survive ~= s1.5 - s3.5   (error ~2e-9)
    birth   ~= s2.5 - s3.5   (error ~4.5e-5)
    out ~= x*(s1.5 - s2.5) + (s2.5 - s3.5)    (max abs err 4.5e-5)

Mapping to engines (per 126-row strip of a 2048x2048 image):
  - DMA: load 128 input rows (1 halo row above/below) as fp32.
  - TensorE: the whole 8-neighbor sum via 3 banded matmuls accumulated in
    PSUM: tridiag-no-center band (vertical neighbors, center column) plus
    full tridiag band applied to column-shifted views (left/right columns);
    W-wrap handled by two 1-column edge matmuls.
  - ScalarE: the three sigmoids straight out of PSUM (scale/bias fused).
  - VectorE/GpSimd: 4 cheap elementwise ops, fp16 (2x DVE mode).
  - DMA out: fp16 (host upcasts to fp32).

Sharding: pure data-parallel over batch: 16 images -> 8 cores x 2 images.
The torus wrap is per-image so there is no cross-core halo at all.
"""

import numpy as np

B, H, W = 16, 2048, 2048
N_CORES = 8
B_PER = B // N_CORES  # 2 images per core
STRIDE = 126  # output rows per strip (128 input rows incl. 1-row halos)
N_STRIPS = (H + STRIDE - 1) // STRIDE  # 17
NBANKS = W // 512  # PSUM banks per strip

_cached_nc = None


def _band_matrices(m, dtype=np.float16):
    """[m+2, m] stationary operands for the vertical taps.

    Tile layout: partitions 0..m-1 hold image rows r0..r0+m-1 (the cells),
    partition m holds the bottom halo row r0+m, partition m+1 holds the top
    halo row r0-1.  For output row p the vertical neighbors are partitions
    p-1 (or m+1 when p==0) and p+1.

    m0[k, p] = 1 for the two vertical neighbors (no center),
    m1[k, p] = 1 for the full 3-tap (used on the column-shifted views).
    """
    m0 = np.zeros((m + 2, m), dtype)
    m1 = np.zeros((m + 2, m), dtype)
    for p in range(m):
        up = m + 1 if p == 0 else p - 1
        m0[up, p] = 1.0
        m0[p + 1, p] = 1.0
        m1[up, p] = 1.0
        m1[p, p] = 1.0
        m1[p + 1, p] = 1.0
    return m0, m1


def _build(b_per=B_PER, h=H, w=W, stride=STRIDE, repeat=1, stages=3):
    global _cached_nc
    if _cached_nc is not None and (b_per, h, w, stride, repeat, stages) == (
        B_PER, H, W, STRIDE, 1, 3
    ):
        return _cached_nc

    import concourse.mybir as mybir
    from concourse.bacc import Bacc
    from concourse.tile import TileContext

    B_PER_, H_, W_, STRIDE_ = b_per, h, w, stride
    N_STRIPS_ = (H_ + STRIDE_ - 1) // STRIDE_
    NBANKS_ = W_ // 512
    KROWS = STRIDE_ + 2  # input rows per full strip

    f32 = mybir.dt.float32
    f16 = mybir.dt.float16
    Sig = mybir.ActivationFunctionType.Sigmoid

    nc = Bacc(trn_type="TRN2")
    x_d = nc.dram_tensor("x", [B_PER_, H_, W_], f32, kind="ExternalInput")
    y_d = nc.dram_tensor("y", [B_PER_, H_, W_], f16, kind="ExternalOutput")

    consts = {}
    for m in sorted({STRIDE_, H_ - STRIDE_ * (N_STRIPS_ - 1)}):
        m0_np, m1_np = _band_matrices(m)
        consts[m] = (
            nc.inline_tensor(m0_np, f"m0_const_{m}"),
            nc.inline_tensor(m1_np, f"m1_const_{m}"),
        )

    with TileContext(nc) as tc:
        with (
            tc.tile_pool(name="wpool", bufs=1) as wpool,
            tc.tile_pool(name="xpool", bufs=6) as xpool,
            tc.tile_pool(name="spool", bufs=5) as spool,
            tc.tile_pool(name="tpool", bufs=5) as tpool,
            tc.tile_pool(name="opool", bufs=6) as opool,
            tc.tile_pool(name="ppool", bufs=2, space="PSUM") as ppool,
        ):
            bands = {}
            for m, (m0_d, m1_d) in consts.items():
                m0 = wpool.tile([m + 2, m], f16, name=f"m0_{m}")
                m1 = wpool.tile([m + 2, m], f16, name=f"m1_{m}")
                nc.sync.dma_start(out=m0[:], in_=m0_d[:])
                nc.sync.dma_start(out=m1[:], in_=m1_d[:])
                bands[m] = (m0, m1)

            # activation biases must be [128,1] APs, not immediates
            b15 = wpool.tile([128, 1], f32)
            b25 = wpool.tile([128, 1], f32)
            b35 = wpool.tile([128, 1], f32)
            nc.vector.memset(b15[:], -15.0)
            nc.vector.memset(b25[:], -25.0)
            nc.vector.memset(b35[:], -35.0)

            for rep in range(repeat):
              for b in range(B_PER_):
                for t in range(N_STRIPS_):
                    r0 = t * STRIDE_  # first output row
                    M = min(STRIDE_, H_ - r0)  # output rows this strip
                    k = M + 2  # partitions used (cells + 2 halos)
                    m0, m1 = bands[M]

                    # fp16 tile, partitions 0..M-1 = cells (rows r0..),
                    # partition M = bottom halo, M+1 = top halo.  gpsimd
                    # (SWDGE) DMA casts fp32->fp16 in flight.
                    xt = xpool.tile([KROWS, W_], f16, tag="xt")
                    if r0 + M < H_:
                        # cells + bottom halo contiguous
                        nc.gpsimd.dma_start(
                            out=xt[0 : M + 1, :], in_=x_d[b, r0 : r0 + M + 1, :]
                        )
                    else:
                        # last strip: bottom halo wraps to row 0
                        nc.gpsimd.dma_start(out=xt[0:M, :], in_=x_d[b, r0:H_, :])
                        nc.gpsimd.dma_start(out=xt[M : M + 1, :], in_=x_d[b, 0:1, :])
                    rtop = (r0 - 1) % H_
                    nc.gpsimd.dma_start(
                        out=xt[M + 1 : M + 2, :], in_=x_d[b, rtop : rtop + 1, :]
                    )

                    ps = ppool.tile([STRIDE_, W_], f32, tag="ps")
                    m0s = m0[:k, :M]
                    m1s = m1[:k, :M]

                    # Pre-touch: a 1x1 matmul absorbs the PSUM-release wait
                    # (Matmult carries at most ONE sync wait; without this,
                    # Bacc's wait-merging couples strip t to strip t-1's
                    # activations and serializes PE behind ACT).
                    nc.tensor.matmul(
                        ps[:1, 0:1], b15[:1, :1], b15[:1, :1],
                        start=True, stop=True,
                    )

                    # around = sum of 8 neighbors, accumulated in PSUM.
                    for nb in range(NBANKS_):
                        c0 = nb * 512
                        c1 = c0 + 512
                        # center column, vertical neighbors only
                        nc.tensor.matmul(
                            ps[:M, c0:c1], m0s, xt[:k, c0:c1],
                            start=True, stop=False,
                        )
                        # left-neighbor column: out col j += band @ x col j-1
                        if nb == 0:
                            nc.tensor.matmul(
                                ps[:M, 1:512], m1s, xt[:k, 0:511],
                                start=False, stop=False,
                            )
                            nc.tensor.matmul(
                                ps[:M, 0:1], m1s, xt[:k, W_ - 1 : W_],
                                start=False, stop=False,
                            )
                        else:
                            nc.tensor.matmul(
                                ps[:M, c0:c1], m1s, xt[:k, c0 - 1 : c1 - 1],
                                start=False, stop=False,
                            )
                        # right-neighbor column: out col j += band @ x col j+1
                        if nb == NBANKS_ - 1:
                            nc.tensor.matmul(
                                ps[:M, c0 : W_ - 1], m1s, xt[:k, c0 + 1 : W_],
                                start=False, stop=False,
                            )
                            nc.tensor.matmul(
                                ps[:M, W_ - 1 : W_], m1s, xt[:k, 0:1],
                                start=False, stop=True,
                            )
                        else:
                            nc.tensor.matmul(
                                ps[:M, c0:c1], m1s, xt[:k, c0 + 1 : c1 + 1],
                                start=False, stop=True,
                            )

                    if stages < 1:
                        continue
                    # one contiguous tile [s15 | s25 | s35] so a single
                    # double-width DVE sub computes d=s15-s25 and e=s25-s35
                    # via overlapping slices
                    sall = spool.tile([STRIDE_, 3 * W_], f16, tag="sall")
                    nc.scalar.activation(sall[:M, 0:W_], ps[:M], Sig, bias=b15[:M], scale=10.0)
                    nc.scalar.activation(sall[:M, W_ : 2 * W_], ps[:M], Sig, bias=b25[:M], scale=10.0)
                    nc.scalar.activation(sall[:M, 2 * W_ : 3 * W_], ps[:M], Sig, bias=b35[:M], scale=10.0)

                    if stages < 2:
                        continue
                    de = tpool.tile([STRIDE_, 2 * W_], f16, tag="de")
                    nc.vector.tensor_sub(
                        out=de[:M], in0=sall[:M, 0 : 2 * W_], in1=sall[:M, W_ : 3 * W_]
                    )
                    if stages >= 3:
                        m = tpool.tile([STRIDE_, W_], f16, tag="m")
                        o = opool.tile([STRIDE_, W_], f16, tag="o")
                        nc.vector.tensor_mul(out=m[:M], in0=xt[:M, :], in1=de[:M, 0:W_])
                        nc.vector.tensor_add(out=o[:M], in0=m[:M], in1=de[:M, W_ : 2 * W_])
                        nc.sync.dma_start(out=y_d[b, r0 : r0 + M, :], in_=o[:M])

    nc.compile()
    if (b_per, h, w, stride, repeat, stages) == (B_PER, H, W, STRIDE, 1, 3):
        _cached_nc = nc
    return nc


def run(x, trace=False):
    """Run the SPMD kernel on 8 cores. Returns (out_fp32, BassKernelResults)."""
    from concourse.bass_utils import run_bass_kernel_spmd

    nc = _build()
    x = np.asarray(x, dtype=np.float32)
    assert x.shape == (B, H, W), x.shape
    in_maps = [{"x": x[B_PER * c : B_PER * (c + 1)]} for c in range(N_CORES)]
    res = run_bass_kernel_spmd(nc, in_maps, core_ids=list(range(N_CORES)), trace=trace)
    out = np.concatenate(
        [res.results[c]["y"].astype(np.float32) for c in range(N_CORES)], axis=0
    )
    return out, res


def kernel(x):
    out, _ = run(x, trace=False)
    return out
